# revision 2
# baseline (speedup 1.0000x reference)
"""Trainium2 Bass kernel: 3x3 VALID conv (NHWC, 256->256 ch) with weight
thresholding + bias, batch-sharded across 8 NeuronCores (4 images/core).

Device strategy per core:
  - x pre-transposed on host to [cin, H*W] (2 partition tiles of 128),
    loaded per image in 4 row-aligned chunks (16 out-rows each) so compute
    starts early and chunks double-buffer.
  - conv = 9 shifted matmuls per output tile accumulated in PSUM over
    9 taps x 2 cin tiles, fp32r (1 cyc/row, TF32-class precision).
  - moving operand is a 3D AP [128, rows, 62] with row stride 64: only the
    62 valid output columns per row are computed (packed output, no
    garbage columns, no padding needed).
  - bias fused into the PSUM->SBUF drain (DVE tensor_scalar_add).
"""

import sys

sys.path.insert(0, "/opt/trn_rl_repo")

import numpy as np

import concourse.bacc as bacc
import concourse.mybir as mybir
import concourse.tile as tile
from concourse.bass_utils import run_bass_kernel_spmd

F32 = mybir.dt.float32
F32R = mybir.dt.float32r

N_CORES = 8
IMG_PER_CORE = 4
C = 256
NPIX = 4096               # 64*64 input pixels per image
NV = 62 * 62              # 3844 valid output pixels per image
# 4 input-row chunks per image: (first_input_row, n_input_rows)
CHUNKS = [(0, 18), (16, 18), (32, 18), (48, 16)]
# output blocks: (out_row0, n_out_rows, chunk_idx)
BLOCKS = [(8 * b, 8 if b < 7 else 6, b // 2) for b in range(8)]
SPARSE_TH = 0.01
TAPS = [(kh, kw) for kh in range(3) for kw in range(3)]

_CACHE = {}


def _declare(nc):
    x_d = nc.dram_tensor("xt", [IMG_PER_CORE, 2, 128, NPIX], F32R,
                         kind="ExternalInput")
    w_d = nc.dram_tensor("wt", [2, 128, 9 * C], F32R, kind="ExternalInput")
    b_d = nc.dram_tensor("bias", [128, 2], F32, kind="ExternalInput")
    o_d = nc.dram_tensor("out", [IMG_PER_CORE, 2, 128, NV], F32,
                         kind="ExternalOutput")
    return x_d, w_d, b_d, o_d


def _emit_prelude(nc, tc, wp, w_d, b_d):
    w_sb = []
    for ct in range(2):
        wt = wp.tile([128, 9 * C], F32R, tag=f"w{ct}")
        nc.sync.dma_start(wt[:], w_d[ct])
        w_sb.append(wt)
    b_sb = wp.tile([128, 2], F32, tag="bias")
    nc.sync.dma_start(b_sb[:], b_d[:])
    return w_sb, b_sb


def _emit_body(nc, tc, xp, pp, op, w_sb, b_sb, x_d, o_d):
    """One rep: conv of IMG_PER_CORE images."""
    for img in range(IMG_PER_CORE):
        x_sb = [[None] * 4 for _ in range(2)]
        for ci, (r0, nr) in enumerate(CHUNKS):
            for ct in range(2):
                xt = xp.tile([128, nr, 64], F32R, tag=f"x{ct}c{ci}")
                nc.sync.dma_start(
                    xt[:], x_d[img, ct, :, r0 * 64:(r0 + nr) * 64])
                x_sb[ct][ci] = xt
        for y0, nrow, ci in BLOCKS:
            n = 62 * nrow
            p0 = 62 * y0
            lr = y0 - CHUNKS[ci][0]
            for co in range(2):
                ps = pp.tile([128, n], F32, tag="ps")
                for ct in range(2):
                    for t, (kh, kw) in enumerate(TAPS):
                        nc.tensor.matmul(
                            ps[:],
                            w_sb[ct][:, t * C + co * 128:
                                     t * C + co * 128 + 128],
                            x_sb[ct][ci][:, lr + kh:lr + kh + nrow,
                                         kw:kw + 62],
                            start=(ct == 0 and t == 0),
                            stop=(ct == 1 and t == 8),
                        )
                ob = op.tile([128, n], F32, tag="ob")
                nc.vector.tensor_scalar_add(
                    ob[:], ps[:], b_sb[:, co:co + 1])
                nc.sync.dma_start(o_d[img, co, :, p0:p0 + n],
                                  ob[:])


def _build(reps: int = 1, hw_loop: bool = False):
    key = (reps, hw_loop)
    if key in _CACHE:
        return _CACHE[key]

    nc = bacc.Bacc("TRN2", target_bir_lowering=False, debug=False,
                   num_devices=N_CORES)
    x_d, w_d, b_d, o_d = _declare(nc)

    with tile.TileContext(nc) as tc:
        with tc.tile_pool(name="wp", bufs=1) as wp, \
             tc.tile_pool(name="xp", bufs=2) as xp, \
             tc.tile_pool(name="pp", bufs=8, space="PSUM") as pp, \
             tc.tile_pool(name="op", bufs=6) as op:
            w_sb, b_sb = _emit_prelude(nc, tc, wp, w_d, b_d)
            if hw_loop:
                with tc.For_i(0, reps):
                    _emit_body(nc, tc, xp, pp, op, w_sb, b_sb, x_d, o_d)
            else:
                for _ in range(reps):
                    _emit_body(nc, tc, xp, pp, op, w_sb, b_sb, x_d, o_d)

    nc.compile()
    _CACHE[key] = nc
    return nc


def _prep_inputs(x, weight, bias):
    """Host-side shard prep: threshold mask + relayout. Per-core in_maps."""
    w = np.where(np.abs(weight) < SPARSE_TH, 0.0, weight).astype(np.float32)
    # (cout, cin, kh, kw) -> (cin, kh, kw, cout) -> [2, 128, 9*256]
    wt = np.ascontiguousarray(w.transpose(1, 2, 3, 0)).reshape(2, 128, 9 * C)
    b2 = np.ascontiguousarray(bias.astype(np.float32).reshape(2, 128).T)

    n_img = x.shape[0]
    xs = np.ascontiguousarray(
        x.astype(np.float32).reshape(n_img, NPIX, C).transpose(0, 2, 1))
    xs = xs.reshape(n_img, 2, 128, NPIX)

    in_maps = []
    for c in range(N_CORES):
        in_maps.append({
            "xt": np.ascontiguousarray(
                xs[c * IMG_PER_CORE:(c + 1) * IMG_PER_CORE]),
            "wt": wt,
            "bias": b2,
        })
    return in_maps


def _assemble(results):
    outs = np.concatenate([r["out"] for r in results], axis=0)  # (32,2,128,3844)
    outs = outs.reshape(32, C, 62, 62).transpose(0, 2, 3, 1)
    return np.ascontiguousarray(outs)


def kernel(x, weight, bias):
    x = np.asarray(x)
    weight = np.asarray(weight)
    bias = np.asarray(bias)
    nc = _build(reps=1)
    in_maps = _prep_inputs(x, weight, bias)
    res = run_bass_kernel_spmd(nc, in_maps, list(range(N_CORES)))
    return _assemble(res.results)


# revision 24
# speedup vs baseline: 12.2973x; 12.2973x over previous
"""Trainium2 Bass kernel: 3x3 VALID conv (NHWC, 256->256 ch) with weight
thresholding + bias, batch-sharded across 8 NeuronCores (4 images/core).

Algorithm: 1D Winograd F(4,3) along W + direct 3-tap conv along H.
Cuts PE moving-column count 553k -> 286k per core vs direct conv.

Per core, per image (x laid out [cin, w, h] in SBUF, fp16):
  - input transform (DVE): for each of 16 w-tiles (stride 4, span 6) build
    6 Winograd points U[u] = B^T d via 8 tensor_tensor + 6
    scalar_tensor_tensor ops, vectorized over (tx, h) [128,16,64] slices.
  - GEMM (PE): M[u] = sum_{kh,cinT} W~[u,kh] @ U[u] shifted by kh: 6-deep
    PSUM accumulation, 496-col fp16 matmuls (FWL weight loads hide under
    streaming).
  - drain (ACT): PSUM -> SBUF fp16 copies; bias folded into the m1 drain
    (A^T column for u=1 is all-ones so every output gets exactly one +b).
  - output transform (DVE): y = A^T m as 6 TT + 3 STT fp16 ops (2x mode).
  - out fp16 [co, (tx,v), h] -> HBM; host casts to fp32 and crops w to 62.

Numerics: fp16 end-to-end with fp32 PSUM accumulation; measured rel err
~4.3e-3 vs fp32 reference (gate 2e-2). bf16 fails (3.5e-2): Winograd's
A^T/B^T amplification needs fp16's 11-bit mantissa.
"""

import sys

sys.path.insert(0, "/opt/trn_rl_repo")

import numpy as np

import concourse.bacc as bacc
import concourse.mybir as mybir
import concourse.tile as tile
from concourse.bass_utils import run_bass_kernel_spmd

F32 = mybir.dt.float32
F16 = mybir.dt.float16
ALU = mybir.AluOpType
ACTF = mybir.ActivationFunctionType

N_CORES = 8
IMG_PER_CORE = 4
C = 256
SPARSE_TH = 0.01

# F(4,3) weight transform (correlation form)
G_MAT = np.array([
    [1 / 4, 0, 0],
    [-1 / 6, -1 / 6, -1 / 6],
    [-1 / 6, 1 / 6, -1 / 6],
    [1 / 24, 1 / 12, 1 / 6],
    [1 / 24, -1 / 12, 1 / 6],
    [0, 0, 1]], dtype=np.float64)

_CACHE = {}


def _declare(nc, internal: bool = False):
    ki = "Internal" if internal else "ExternalInput"
    ko = "Internal" if internal else "ExternalOutput"
    # x: [img, cinT, ci, w, h] fp16
    x_d = nc.dram_tensor("xw", [IMG_PER_CORE, 2, 128, 64, 64], F16, kind=ki)
    # Winograd-transformed weights: [cinT, ci, u, kh, coT, co] fp16
    w_d = nc.dram_tensor("ww", [2, 128, 6, 3, 2, 128], F16, kind=ki)
    b_d = nc.dram_tensor("bias", [128, 2], F32, kind=ki)
    # out: [img, coT, co, tx, v, h] fp16  (w = 4*tx + v; w=62,63 garbage)
    o_d = nc.dram_tensor("out", [IMG_PER_CORE, 2, 128, 16, 4, 62], F16,
                         kind=ko)
    return x_d, w_d, b_d, o_d


def _emit_prelude(nc, tc, wp, w_d, b_d):
    w_sb = []
    for ct in range(2):
        wt = wp.tile([128, 6, 3, 2, 128], F16, tag=f"w{ct}")
        nc.sync.dma_start(wt[:], w_d[ct])
        w_sb.append(wt)
    b_sb = wp.tile([128, 2], F32, tag="bias")
    nc.sync.dma_start(b_sb[:], b_d[:])
    return w_sb, b_sb


def _emit_body(nc, tc, xp, up, tp, pp, mp, yp, w_sb, b_sb, x_d, o_d):
    """One rep, software-pipelined: transform img i+1 is emitted before the
    GEMM of img i so the DVE FIFO never head-of-line blocks the PE."""
    v = nc.vector

    def load_and_transform(img):
        # ---- load x [ci, w(64)+2 pad, h] and zero the pad columns ----
        x_sb = []
        for ct in range(2):
            xt = xp.tile([128, 66, 64], F16, tag=f"x{ct}")
            nc.sync.dma_start(xt[:, 0:64, :], x_d[img, ct])
            nc.gpsimd.memset(xt[:, 64:66, :], 0.0)
            x_sb.append(xt)

        # ---- input transform: U[u] = B^T d over all (tx, h) ----
        u_sb = []
        for ct in range(2):
            xt = x_sb[ct]
            d = [xt[:, j:j + 61:4, :] for j in range(6)]  # [128,16,64] each
            ut = up.tile([128, 16, 6, 64], F16, tag=f"u{ct}")

            def tmp(tag):
                return tp.tile([128, 16, 64], F16, tag=tag, name=tag)

            # subexpr TTs on DVE (2x f16); scale-muls on the idle scalar
            # engine (Identity: out = scale*in); u-point adds back on DVE.
            # STT would fuse these but runs 1x (no 2x uop) - slower.
            e = tmp("e"); v.tensor_sub(e[:], d[3], d[1])
            f = tmp("f"); v.tensor_sub(f[:], d[4], d[2])
            g = tmp("g"); v.tensor_sub(g[:], d[0], d[2])
            h = tmp("h"); v.tensor_sub(h[:], d[5], d[3])
            s = tmp("s"); v.tensor_add(s[:], d[1], d[2])
            t = tmp("t"); v.tensor_add(t[:], d[3], d[4])
            p = tmp("p"); v.tensor_sub(p[:], d[1], d[2])
            q = tmp("q"); v.tensor_sub(q[:], d[4], d[3])
            g4 = tmp("g4"); v.tensor_scalar_mul(g4[:], g[:], 4.0)
            s4 = tmp("s4"); nc.scalar.activation(s4[:], s[:], ACTF.Identity,
                                                 scale=-4.0)
            p4 = tmp("p4"); nc.scalar.activation(p4[:], p[:], ACTF.Identity,
                                                 scale=4.0)
            e2 = tmp("e2"); nc.scalar.activation(e2[:], e[:], ACTF.Identity,
                                                 scale=2.0)
            e4 = tmp("e4"); nc.scalar.activation(e4[:], e[:], ACTF.Identity,
                                                 scale=-4.0)
            v.tensor_add(ut[:, :, 0, :], g4[:], f[:])
            v.tensor_add(ut[:, :, 1, :], s4[:], t[:])
            v.tensor_add(ut[:, :, 2, :], p4[:], q[:])
            v.tensor_add(ut[:, :, 3, :], e2[:], f[:])
            v.tensor_sub(ut[:, :, 4, :], f[:], e2[:])
            v.tensor_add(ut[:, :, 5, :], e4[:], h[:])
            u_sb.append(ut)
        return u_sb

    def gemm_and_out(img, u_sb):
        # ---- GEMM + drain + output transform, per (coT, tx-chunk) ----
        for coT in range(2):
            for txc in range(2):
                tx0 = txc * 8
                # u0, u1 in single-bank tiles (u1 drain carries the bias);
                # u2/u3 and u4/u5 pair into 2-bank PSUM tiles so one ACT op
                # drains both (PSUM is linear per partition across banks).
                m = []
                ps_of = {}
                for u in range(6):
                    if u < 2:
                        ps = pp.tile([128, 8, 62], F32, tag=f"ps{u}",
                                     name=f"ps{u}", bufs=2)
                        ps_of[u] = ps[:]
                    elif u % 2 == 0:
                        # padded so each half is bank-aligned (matmul out
                        # must stay within one 2KB PSUM bank)
                        ps2 = pp.tile([128, 2, 8, 62], F32, tag=f"ps{u}p",
                                      name=f"ps{u}p",
                                      padded_shape=[128, 2, 8, 64])
                        ps_of[u] = ps2[:, 0]
                        ps_of[u + 1] = ps2[:, 1]
                    k = 0
                    for ct in range(2):
                        for kh in range(3):
                            nc.tensor.matmul(
                                ps_of[u],
                                w_sb[ct][:, u, kh, coT, :],
                                u_sb[ct][:, tx0:tx0 + 8, u, kh:kh + 62],
                                start=(k == 0), stop=(k == 5))
                            k += 1
                    if u == 0:
                        mt = mp.tile([128, 8, 62], F16, tag="m0", name="m0")
                        nc.scalar.activation(mt[:], ps_of[0], ACTF.Identity)
                        m.append(mt)
                    elif u == 1:
                        mt = mp.tile([128, 8, 62], F16, tag="m1", name="m1")
                        nc.scalar.activation(mt[:], ps_of[1], ACTF.Identity,
                                             bias=b_sb[:, coT:coT + 1])
                        m.append(mt)
                    elif u % 2 == 1:
                        mt = mp.tile([128, 2, 8, 62], F16, tag=f"m{u}p",
                                     name=f"m{u}p")
                        nc.scalar.activation(mt[:], ps2[:], ACTF.Identity)
                        m.append(mt[:, 0])
                        m.append(mt[:, 1])

                def otmp(tag):
                    return tp.tile([128, 8, 62], F16, tag=tag, name=tag)

                s2 = otmp("s2"); v.tensor_add(s2[:], m[1][:], m[2][:])
                t2 = otmp("t2"); v.tensor_sub(t2[:], m[1][:], m[2][:])
                p2 = otmp("p2"); v.tensor_add(p2[:], m[3][:], m[4][:])
                q2 = otmp("q2"); v.tensor_sub(q2[:], m[3][:], m[4][:])
                a0 = otmp("a0"); v.tensor_add(a0[:], m[0][:], s2[:])
                q2m = otmp("q2m"); nc.scalar.activation(q2m[:], q2[:],
                                                        ACTF.Identity,
                                                        scale=2.0)
                p4m = otmp("p4m"); nc.scalar.activation(p4m[:], p2[:],
                                                        ACTF.Identity,
                                                        scale=4.0)
                q8m = otmp("q8m"); nc.scalar.activation(q8m[:], q2[:],
                                                        ACTF.Identity,
                                                        scale=8.0)
                yt = yp.tile([128, 8, 4, 62], F16, tag="y")
                v.tensor_add(yt[:, :, 0, :], a0[:], p2[:])
                v.tensor_add(yt[:, :, 1, :], q2m[:], t2[:])
                v.tensor_add(yt[:, :, 2, :], p4m[:], s2[:])
                y3 = otmp("y3")
                v.tensor_add(y3[:], q8m[:], t2[:])
                v.tensor_add(yt[:, :, 3, :], y3[:], m[5][:])
                nc.sync.dma_start(o_d[img, coT, :, tx0:tx0 + 8, :, :], yt[:])

    for img in range(IMG_PER_CORE):
        u_sb = load_and_transform(img)
        gemm_and_out(img, u_sb)


def _build(reps: int = 1, hw_loop: bool = False, internal: bool = False):
    key = (reps, hw_loop, internal)
    if key in _CACHE:
        return _CACHE[key]

    nc = bacc.Bacc("TRN2", target_bir_lowering=False, debug=False,
                   num_devices=N_CORES)
    x_d, w_d, b_d, o_d = _declare(nc, internal=internal)

    with tile.TileContext(nc) as tc:
        with tc.tile_pool(name="wp", bufs=1) as wp, \
             tc.tile_pool(name="xp", bufs=2) as xp, \
             tc.tile_pool(name="up", bufs=2) as up, \
             tc.tile_pool(name="tp", bufs=2) as tp, \
             tc.tile_pool(name="pp", bufs=1, space="PSUM") as pp, \
             tc.tile_pool(name="mp", bufs=2) as mp, \
             tc.tile_pool(name="yp", bufs=2) as yp:
            w_sb, b_sb = _emit_prelude(nc, tc, wp, w_d, b_d)
            if hw_loop:
                with tc.For_i(0, reps, staggered_reset=True):
                    _emit_body(nc, tc, xp, up, tp, pp, mp, yp,
                               w_sb, b_sb, x_d, o_d)
            else:
                for _ in range(reps):
                    _emit_body(nc, tc, xp, up, tp, pp, mp, yp,
                               w_sb, b_sb, x_d, o_d)

    nc.compile()
    _CACHE[key] = nc
    return nc


def _prep_inputs(x, weight, bias):
    """Host-side prep: threshold mask, Winograd weight transform, relayout."""
    w = np.where(np.abs(weight) < SPARSE_TH, 0.0, weight).astype(np.float64)
    # W~[u, co, ci, kh] = sum_kw G[u,kw] w[co,ci,kh,kw]
    wt = np.einsum("uk,oihk->uoih", G_MAT, w)
    # -> [ci, u, kh, co] -> [cinT, 128, u, kh, coT, co]
    wt = np.ascontiguousarray(wt.transpose(2, 0, 3, 1)).reshape(
        2, 128, 6, 3, 2, 128).astype(np.float16)
    b2 = np.ascontiguousarray(
        bias.astype(np.float32).reshape(2, 128).T)

    n_img = x.shape[0]
    # [n, h, w, c] -> [n, c, w, h] -> [n, cinT, 128, 64, 64]
    xs = np.ascontiguousarray(
        x.transpose(0, 3, 2, 1).astype(np.float16)).reshape(
        n_img, 2, 128, 64, 64)

    in_maps = []
    for c in range(N_CORES):
        in_maps.append({
            "xw": np.ascontiguousarray(
                xs[c * IMG_PER_CORE:(c + 1) * IMG_PER_CORE]),
            "ww": wt,
            "bias": b2,
        })
    return in_maps


def _assemble(results):
    outs = np.concatenate([r["out"] for r in results], axis=0)
    # [32, coT, co, tx, v, h] fp16 -> [32, 2, 128, 64, 62] fp32
    outs = outs.astype(np.float32).reshape(32, 2, 128, 64, 62)
    # -> [n, h, w, coT, co] -> [n, 62, 62, 256]
    outs = outs.transpose(0, 4, 3, 1, 2).reshape(32, 62, 64, C)[:, :, :62, :]
    return np.ascontiguousarray(outs)


def kernel(x, weight, bias):
    x = np.asarray(x)
    weight = np.asarray(weight)
    bias = np.asarray(bias)
    nc = _build(reps=1)
    in_maps = _prep_inputs(x, weight, bias)
    res = run_bass_kernel_spmd(nc, in_maps, list(range(N_CORES)))
    return _assemble(res.results)


# revision 25
# speedup vs baseline: 19.9561x; 1.6228x over previous
"""Trainium2 Bass kernel: 3x3 VALID conv (NHWC, 256->256 ch) with weight
thresholding + bias, batch-sharded across 8 NeuronCores (4 images/core).

Algorithm: 1D Winograd F(4,3) along W + direct 3-tap conv along H.
Cuts PE moving-column count 553k -> 286k per core vs direct conv.

Per core, per image (x laid out [cin, w, h] in SBUF, fp16):
  - input transform: for each of 16 w-tiles (stride 4, span 6) build the 6
    Winograd points U[u] = B^T d, vectorized over (tx, h) [128,16,64]
    slices. Two-input adds run on DVE tensor_tensor (2x_1P fp16 mode,
    594ns); the x4/x2 scale-muls run on the otherwise-idle scalar engine
    (Identity activation, out = scale*in) except one on DVE tensor_scalar
    to balance engine load. scalar_tensor_tensor would fuse scale+add in
    one op but only has a 1x uop (1127ns) - slower than the split.
  - GEMM (PE): M[u] = sum_{kh,cinT} W~[u,kh] @ U[u] shifted by kh: 6-deep
    PSUM accumulation, 496-col fp16 matmuls; u2/u3 and u4/u5 share 2-bank
    PSUM tiles (bank-aligned halves) so one ACT op drains both.
  - drain (ACT): PSUM -> SBUF fp16; bias folded into the m1 drain (A^T
    column for u=1 is all-ones so every output gets exactly one +b).
  - output transform (DVE + ACT scale-muls): y = A^T m, all fp16 2x TTs.
  - out fp16 [co, (tx,v), h] -> HBM; host casts to fp32 and crops w to 62.

Engine balance per rep (sim): PE 120us (100% busy steady-state), DVE
~110us, ACT ~105us; gpsimd only does memsets - its elementwise ops are
~5x slower than the cost model claims on real HW.

Numerics: fp16 end-to-end with fp32 PSUM accumulation; measured rel err
~4.3e-3 vs fp32 reference (gate 2e-2). bf16 fails (3.5e-2): Winograd's
A^T/B^T amplification needs fp16's 11-bit mantissa.

Measured (For_i rep-amplified, 8 reps/iteration, quiet machine): ~140us
per rep vs 285us direct-conv fp32r baseline (~2x). Machine-load drift of
+-15% affects absolute numbers run to run.
"""

import sys

sys.path.insert(0, "/opt/trn_rl_repo")

import numpy as np

import concourse.bacc as bacc
import concourse.mybir as mybir
import concourse.tile as tile
from concourse.bass_utils import run_bass_kernel_spmd

F32 = mybir.dt.float32
F16 = mybir.dt.float16
ALU = mybir.AluOpType
ACTF = mybir.ActivationFunctionType

N_CORES = 8
IMG_PER_CORE = 4
C = 256
SPARSE_TH = 0.01

# F(4,3) weight transform (correlation form)
G_MAT = np.array([
    [1 / 4, 0, 0],
    [-1 / 6, -1 / 6, -1 / 6],
    [-1 / 6, 1 / 6, -1 / 6],
    [1 / 24, 1 / 12, 1 / 6],
    [1 / 24, -1 / 12, 1 / 6],
    [0, 0, 1]], dtype=np.float64)

_CACHE = {}


def _declare(nc, internal: bool = False):
    ki = "Internal" if internal else "ExternalInput"
    ko = "Internal" if internal else "ExternalOutput"
    # x: [img, cinT, ci, w, h] fp16
    x_d = nc.dram_tensor("xw", [IMG_PER_CORE, 2, 128, 64, 64], F16, kind=ki)
    # Winograd-transformed weights: [cinT, ci, u, kh, coT, co] fp16
    w_d = nc.dram_tensor("ww", [2, 128, 6, 3, 2, 128], F16, kind=ki)
    b_d = nc.dram_tensor("bias", [128, 2], F32, kind=ki)
    # out: [img, coT, co, tx, v, h] fp16  (w = 4*tx + v; w=62,63 garbage)
    o_d = nc.dram_tensor("out", [IMG_PER_CORE, 2, 128, 16, 4, 62], F16,
                         kind=ko)
    return x_d, w_d, b_d, o_d


def _emit_prelude(nc, tc, wp, w_d, b_d):
    w_sb = []
    for ct in range(2):
        wt = wp.tile([128, 6, 3, 2, 128], F16, tag=f"w{ct}")
        nc.sync.dma_start(wt[:], w_d[ct])
        w_sb.append(wt)
    b_sb = wp.tile([128, 2], F32, tag="bias")
    nc.sync.dma_start(b_sb[:], b_d[:])
    return w_sb, b_sb


def _emit_body(nc, tc, xp, up, tp, pp, mp, yp, w_sb, b_sb, x_d, o_d):
    """One rep, software-pipelined: transform img i+1 is emitted before the
    GEMM of img i so the DVE FIFO never head-of-line blocks the PE."""
    v = nc.vector

    def load_and_transform(img):
        # ---- load x [ci, w(64)+2 pad, h] and zero the pad columns ----
        x_sb = []
        for ct in range(2):
            xt = xp.tile([128, 66, 64], F16, tag=f"x{ct}")
            nc.sync.dma_start(xt[:, 0:64, :], x_d[img, ct])
            nc.gpsimd.memset(xt[:, 64:66, :], 0.0)
            x_sb.append(xt)

        # ---- input transform: U[u] = B^T d over all (tx, h) ----
        u_sb = []
        for ct in range(2):
            xt = x_sb[ct]
            d = [xt[:, j:j + 61:4, :] for j in range(6)]  # [128,16,64] each
            ut = up.tile([128, 16, 6, 64], F16, tag=f"u{ct}")

            def tmp(tag):
                return tp.tile([128, 16, 64], F16, tag=tag, name=tag)

            # subexpr TTs on DVE (2x f16); scale-muls on the idle scalar
            # engine (Identity: out = scale*in); u-point adds back on DVE.
            # STT would fuse these but runs 1x (no 2x uop) - slower.
            e = tmp("e"); v.tensor_sub(e[:], d[3], d[1])
            f = tmp("f"); v.tensor_sub(f[:], d[4], d[2])
            g = tmp("g"); v.tensor_sub(g[:], d[0], d[2])
            h = tmp("h"); v.tensor_sub(h[:], d[5], d[3])
            s = tmp("s"); v.tensor_add(s[:], d[1], d[2])
            t = tmp("t"); v.tensor_add(t[:], d[3], d[4])
            p = tmp("p"); v.tensor_sub(p[:], d[1], d[2])
            q = tmp("q"); v.tensor_sub(q[:], d[4], d[3])
            g4 = tmp("g4"); v.tensor_scalar_mul(g4[:], g[:], 4.0)
            s4 = tmp("s4"); nc.scalar.activation(s4[:], s[:], ACTF.Identity,
                                                 scale=-4.0)
            p4 = tmp("p4"); nc.scalar.activation(p4[:], p[:], ACTF.Identity,
                                                 scale=4.0)
            e2 = tmp("e2"); nc.scalar.activation(e2[:], e[:], ACTF.Identity,
                                                 scale=2.0)
            e4 = tmp("e4"); nc.scalar.activation(e4[:], e[:], ACTF.Identity,
                                                 scale=-4.0)
            v.tensor_add(ut[:, :, 0, :], g4[:], f[:])
            v.tensor_add(ut[:, :, 1, :], s4[:], t[:])
            v.tensor_add(ut[:, :, 2, :], p4[:], q[:])
            v.tensor_add(ut[:, :, 3, :], e2[:], f[:])
            v.tensor_sub(ut[:, :, 4, :], f[:], e2[:])
            v.tensor_add(ut[:, :, 5, :], e4[:], h[:])
            u_sb.append(ut)
        return u_sb

    def gemm_and_out(img, u_sb):
        # ---- GEMM + drain + output transform, per (coT, tx-chunk) ----
        for coT in range(2):
            for txc in range(2):
                tx0 = txc * 8
                # u0, u1 in single-bank tiles (u1 drain carries the bias);
                # u2/u3 and u4/u5 pair into 2-bank PSUM tiles so one ACT op
                # drains both (PSUM is linear per partition across banks).
                m = []
                ps_of = {}
                for u in range(6):
                    if u < 2:
                        ps = pp.tile([128, 8, 62], F32, tag=f"ps{u}",
                                     name=f"ps{u}", bufs=2)
                        ps_of[u] = ps[:]
                    elif u % 2 == 0:
                        # padded so each half is bank-aligned (matmul out
                        # must stay within one 2KB PSUM bank)
                        ps2 = pp.tile([128, 2, 8, 62], F32, tag=f"ps{u}p",
                                      name=f"ps{u}p",
                                      padded_shape=[128, 2, 8, 64])
                        ps_of[u] = ps2[:, 0]
                        ps_of[u + 1] = ps2[:, 1]
                    k = 0
                    for ct in range(2):
                        for kh in range(3):
                            nc.tensor.matmul(
                                ps_of[u],
                                w_sb[ct][:, u, kh, coT, :],
                                u_sb[ct][:, tx0:tx0 + 8, u, kh:kh + 62],
                                start=(k == 0), stop=(k == 5))
                            k += 1
                    if u == 0:
                        mt = mp.tile([128, 8, 62], F16, tag="m0", name="m0")
                        nc.scalar.activation(mt[:], ps_of[0], ACTF.Identity)
                        m.append(mt)
                    elif u == 1:
                        mt = mp.tile([128, 8, 62], F16, tag="m1", name="m1")
                        nc.scalar.activation(mt[:], ps_of[1], ACTF.Identity,
                                             bias=b_sb[:, coT:coT + 1])
                        m.append(mt)
                    elif u % 2 == 1:
                        mt = mp.tile([128, 2, 8, 62], F16, tag=f"m{u}p",
                                     name=f"m{u}p")
                        nc.scalar.activation(mt[:], ps2[:], ACTF.Identity)
                        m.append(mt[:, 0])
                        m.append(mt[:, 1])

                def otmp(tag):
                    return tp.tile([128, 8, 62], F16, tag=tag, name=tag)

                s2 = otmp("s2"); v.tensor_add(s2[:], m[1][:], m[2][:])
                t2 = otmp("t2"); v.tensor_sub(t2[:], m[1][:], m[2][:])
                p2 = otmp("p2"); v.tensor_add(p2[:], m[3][:], m[4][:])
                q2 = otmp("q2"); v.tensor_sub(q2[:], m[3][:], m[4][:])
                a0 = otmp("a0"); v.tensor_add(a0[:], m[0][:], s2[:])
                q2m = otmp("q2m"); nc.scalar.activation(q2m[:], q2[:],
                                                        ACTF.Identity,
                                                        scale=2.0)
                p4m = otmp("p4m"); nc.scalar.activation(p4m[:], p2[:],
                                                        ACTF.Identity,
                                                        scale=4.0)
                q8m = otmp("q8m"); nc.scalar.activation(q8m[:], q2[:],
                                                        ACTF.Identity,
                                                        scale=8.0)
                yt = yp.tile([128, 8, 4, 62], F16, tag="y")
                v.tensor_add(yt[:, :, 0, :], a0[:], p2[:])
                v.tensor_add(yt[:, :, 1, :], q2m[:], t2[:])
                v.tensor_add(yt[:, :, 2, :], p4m[:], s2[:])
                y3 = otmp("y3")
                v.tensor_add(y3[:], q8m[:], t2[:])
                v.tensor_add(yt[:, :, 3, :], y3[:], m[5][:])
                nc.sync.dma_start(o_d[img, coT, :, tx0:tx0 + 8, :, :], yt[:])

    for img in range(IMG_PER_CORE):
        u_sb = load_and_transform(img)
        gemm_and_out(img, u_sb)


def _build(reps: int = 1, hw_loop: bool = False, internal: bool = False):
    key = (reps, hw_loop, internal)
    if key in _CACHE:
        return _CACHE[key]

    nc = bacc.Bacc("TRN2", target_bir_lowering=False, debug=False,
                   num_devices=N_CORES)
    x_d, w_d, b_d, o_d = _declare(nc, internal=internal)

    with tile.TileContext(nc) as tc:
        with tc.tile_pool(name="wp", bufs=1) as wp, \
             tc.tile_pool(name="xp", bufs=2) as xp, \
             tc.tile_pool(name="up", bufs=2) as up, \
             tc.tile_pool(name="tp", bufs=2) as tp, \
             tc.tile_pool(name="pp", bufs=1, space="PSUM") as pp, \
             tc.tile_pool(name="mp", bufs=2) as mp, \
             tc.tile_pool(name="yp", bufs=2) as yp:
            w_sb, b_sb = _emit_prelude(nc, tc, wp, w_d, b_d)
            if hw_loop:
                with tc.For_i(0, reps, staggered_reset=True):
                    _emit_body(nc, tc, xp, up, tp, pp, mp, yp,
                               w_sb, b_sb, x_d, o_d)
            else:
                for _ in range(reps):
                    _emit_body(nc, tc, xp, up, tp, pp, mp, yp,
                               w_sb, b_sb, x_d, o_d)

    nc.compile()
    _CACHE[key] = nc
    return nc


def _prep_inputs(x, weight, bias):
    """Host-side prep: threshold mask, Winograd weight transform, relayout."""
    w = np.where(np.abs(weight) < SPARSE_TH, 0.0, weight).astype(np.float64)
    # W~[u, co, ci, kh] = sum_kw G[u,kw] w[co,ci,kh,kw]
    wt = np.einsum("uk,oihk->uoih", G_MAT, w)
    # -> [ci, u, kh, co] -> [cinT, 128, u, kh, coT, co]
    wt = np.ascontiguousarray(wt.transpose(2, 0, 3, 1)).reshape(
        2, 128, 6, 3, 2, 128).astype(np.float16)
    b2 = np.ascontiguousarray(
        bias.astype(np.float32).reshape(2, 128).T)

    n_img = x.shape[0]
    # [n, h, w, c] -> [n, c, w, h] -> [n, cinT, 128, 64, 64]
    xs = np.ascontiguousarray(
        x.transpose(0, 3, 2, 1).astype(np.float16)).reshape(
        n_img, 2, 128, 64, 64)

    in_maps = []
    for c in range(N_CORES):
        in_maps.append({
            "xw": np.ascontiguousarray(
                xs[c * IMG_PER_CORE:(c + 1) * IMG_PER_CORE]),
            "ww": wt,
            "bias": b2,
        })
    return in_maps


def _assemble(results):
    outs = np.concatenate([r["out"] for r in results], axis=0)
    # [32, coT, co, tx, v, h] fp16 -> [32, 2, 128, 64, 62] fp32
    outs = outs.astype(np.float32).reshape(32, 2, 128, 64, 62)
    # -> [n, h, w, coT, co] -> [n, 62, 62, 256]
    outs = outs.transpose(0, 4, 3, 1, 2).reshape(32, 62, 64, C)[:, :, :62, :]
    return np.ascontiguousarray(outs)


def kernel(x, weight, bias):
    x = np.asarray(x)
    weight = np.asarray(weight)
    bias = np.asarray(bias)
    nc = _build(reps=1)
    in_maps = _prep_inputs(x, weight, bias)
    res = run_bass_kernel_spmd(nc, in_maps, list(range(N_CORES)))
    return _assemble(res.results)
